# revision 34
# baseline (speedup 1.0000x reference)
"""MQA kernel for Trainium2 (8 NeuronCores, SPMD via bass/Tile).

Problem: nn_MultiQueryAttention (B=2, T=2048, HID=2048, H=16, D=128).

Key algebraic simplification: the reference's apply_rope treats q's layout
as (B,T,H,D) while q is actually (B,H,T,D), so the "position" axis is the
head index -> per-head rotation R_h acting on the D axis only, independent
of sequence position. R_h is folded into Wq on the host. k's rope at pos=0
is a pure channel permutation, folded into Wk. The score scale 1/sqrt(D)
is folded into Wq as well. What remains on-device is a plain causal MQA.

Sharding (uniform SPMD program, per-core data differs):
  core c -> batch c//4, heads (c%4)*4..(c%4)*4+3, full T.
  Each core: Q^T/K^T/V projections, causal softmax attention for its 4
  heads, and a partial out-projection (its heads' rows of Wo^T). The 4
  partials per batch are summed on the host.

v2 (from the 348us fp32r baseline, tensor-engine bound at 84%):
  * bf16 operands everywhere (PE streams 1 cycle/row either way, but DMA
    and SBUF halve and DVE gets 2x/4x modes). PSUM accumulation stays fp32.
  * softmax denominators off the PE: the old ones-matmul (40us) and
    reciprocal-broadcast matmul (7.5us) are replaced by bf16 DVE
    accumulation of exp tiles plus one GpSimd partition_all_reduce per
    query tile (idle engine), which also broadcasts the sums across
    partitions for free.
  * out-projection matmuls merged to 1024-wide (256 -> 128 instrs), with
    PSUM drains alternating between DVE and ACT.
"""

import numpy as np
from contextlib import ExitStack

# Correct the compile-time cost model BEFORE any other concourse import: the
# Rust scheduler caches hw specs in a process-wide OnceLock on first use, and
# stock GPSIMD_IMPL_EFFICIENCY prices PartitionAllReduce ~4x too fast
# (measured 3.53us for a [128,512] tile => efficiency ~0.12, not 0.60). With
# the wrong number the Tile list-scheduler statically orders each softmax
# row's reciprocal right behind its reduce, serializing every row on the
# 3.5us GpSimd latency. Scheduling knob only — emitted math is unchanged.
import concourse.hw_specs as _hw_specs
_hw_specs.TRN2Spec.GPSIMD_IMPL_EFFICIENCY = {
    **_hw_specs.TRN2Spec.GPSIMD_IMPL_EFFICIENCY, "PartitionAllReduce": 0.12}

import concourse.bass as bass
import concourse.tile as tile
from concourse import bacc, bass_isa, mybir
from concourse.bass_utils import run_bass_kernel_spmd
from concourse.masks import make_identity

F32 = mybir.dt.float32
BF16 = mybir.dt.bfloat16
EXP = mybir.ActivationFunctionType.Exp
COPY = mybir.ActivationFunctionType.Copy

B, T, HID, H, D = 2, 2048, 2048, 16, 128
NCORES = 8
CPB = 4              # cores per batch
HPC = H // CPB       # 4 heads per core
HD_PC = HPC * D      # 512 output dims per core
P = 128
KT = T // P          # 16 key tiles
NK = HID // P        # 16 contraction tiles for projections


def _rope_fold():
    """Per-head rotation matrices R_h (128x128) from the reference's quirky rope."""
    half = D // 2
    theta = 1.0 / (10000.0 ** (np.arange(0, half, 2, dtype=np.float64) / half))
    mats = []
    for h in range(H):
        R = np.zeros((D, D), dtype=np.float64)
        c = np.cos(h * theta)
        s = np.sin(h * theta)
        for j in range(32):
            R[j, 2 * j] = c[j]
            R[j, 2 * j + 1] = -s[j]
            R[32 + j, 2 * j] = s[j]
            R[32 + j, 2 * j + 1] = c[j]
            R[64 + j, 64 + 2 * j] = c[j]
            R[64 + j, 64 + 2 * j + 1] = -s[j]
            R[96 + j, 64 + 2 * j] = s[j]
            R[96 + j, 64 + 2 * j + 1] = c[j]
        mats.append(R)
    return mats


def _build_program():
    nc = bacc.Bacc("TRN2", target_bir_lowering=False, debug=False,
                   enable_asserts=False, num_devices=NCORES)

    hsT = nc.dram_tensor("hsT", [HID, T], BF16, kind="ExternalInput").ap()
    wqT = nc.dram_tensor("wqT", [HID, HD_PC], BF16, kind="ExternalInput").ap()
    wkT = nc.dram_tensor("wkT", [HID, D], BF16, kind="ExternalInput").ap()
    wvT = nc.dram_tensor("wvT", [HID, D], BF16, kind="ExternalInput").ap()
    woT = nc.dram_tensor("woT", [HD_PC, HID], BF16, kind="ExternalInput").ap()
    dmd = nc.dram_tensor("dmask", [P, P], BF16, kind="ExternalInput").ap()
    out = nc.dram_tensor("out", [T, HID], BF16, kind="ExternalOutput").ap()

    hsT_r = hsT.rearrange("(ko p) t -> ko p t", p=P)        # [16,128,2048]
    wqT_r = wqT.rearrange("(ko p) m -> p ko m", p=P)        # [128,16,512]
    wkT_r = wkT.rearrange("(ko p) d -> p ko d", p=P)        # [128,16,128]
    wvT_r = wvT.rearrange("(ko p) d -> p ko d", p=P)
    woT_r = woT.rearrange("(h p) n -> p h n", p=P)          # [128,4,2048]
    out_r = out.rearrange("(tt p) n -> tt p n", p=P)        # [16,128,2048]

    def mm(ps, lhsT, rhs, start, stop):
        nc.tensor.matmul(ps, lhsT=lhsT, rhs=rhs, start=start, stop=stop)

    with tile.TileContext(nc) as tc, ExitStack() as ctx:
        singles = ctx.enter_context(tc.tile_pool(name="singles", bufs=1))
        hpool = ctx.enter_context(tc.tile_pool(name="hst", bufs=8))
        epool = ctx.enter_context(tc.tile_pool(name="etile", bufs=6))
        spool = ctx.enter_context(tc.tile_pool(name="small", bufs=2))
        apool = ctx.enter_context(tc.tile_pool(name="accp", bufs=3))
        dpool = ctx.enter_context(tc.tile_pool(name="denp", bufs=3))
        opool = ctx.enter_context(tc.tile_pool(name="outt", bufs=4))

        ident = singles.tile([P, P], BF16)
        make_identity(nc, ident)
        dmask = singles.tile([P, P], BF16)

        # weight residents; per-k slices are DMA'd inside the first
        # phase-1 block so the first matmuls start after ~1us, and the
        # out-projection weights load during attention.
        wq_sb = singles.tile([P, NK, HD_PC], BF16)
        wk_sb = singles.tile([P, NK, D], BF16)
        wv_sb = singles.tile([P, NK, D], BF16)
        wo_sb = singles.tile([P, HPC, HID], BF16)

        # resident activations
        qt_sb = singles.tile([P, HPC, T], BF16)      # Q^T per head [d, t]
        kt_sb = singles.tile([P, T], BF16)           # K^T [d, s]
        v_sb = singles.tile([P, KT, D], BF16)        # V natural [s-tile, d]
        at_sb = singles.tile([P, HPC, T], BF16)      # normalized O^T per head

        # ---------------- Phase 1: Q/K/V projections ----------------
        # V^T -> V natural transposes are emitted one t-block late so the
        # PE never stalls on the DVE psum->sbuf cast of the current block.
        with tc.tile_pool(name="ps1", bufs=1, space="PSUM") as ps1, \
             tc.tile_pool(name="ps1t", bufs=2, space="PSUM") as ps1t:
            pend_vt = None
            for tb4 in range(4):           # 512-wide t blocks
                tsl = slice(tb4 * 512, (tb4 + 1) * 512)
                q_ps = [ps1.tile([P, 512], F32, tag=f"qps{h}", name=f"qps{h}")
                        for h in range(HPC)]
                k_ps = ps1.tile([P, 512], F32, tag="kps")
                v_ps = ps1.tile([P, 512], F32, tag="vps")
                for k in range(NK):
                    if tb4 == 0:  # stream weight slices in with the data
                        nc.sync.dma_start(out=wk_sb[:, k, :], in_=wkT_r[:, k, :])
                        nc.sync.dma_start(out=wv_sb[:, k, :], in_=wvT_r[:, k, :])
                        nc.sync.dma_start(out=wq_sb[:, k, :], in_=wqT_r[:, k, :])
                        if k == 8:
                            nc.sync.dma_start(out=dmask, in_=dmd)
                        if k >= 12:  # out-proj weights: first 4 of 16 chunks
                            c = k - 12
                            csl = slice((c % 4) * 512, (c % 4 + 1) * 512)
                            nc.sync.dma_start(out=wo_sb[:, c // 4, csl],
                                              in_=woT_r[:, c // 4, csl])
                    elif tb4 == 1 and k < 12:
                        # remaining out-proj chunks, 128KB each so no DMA
                        # queue is ever pinned behind a half-megabyte load
                        # while the activation stream needs it
                        c = 4 + k
                        csl = slice((c % 4) * 512, (c % 4 + 1) * 512)
                        nc.sync.dma_start(out=wo_sb[:, c // 4, csl],
                                          in_=woT_r[:, c // 4, csl])
                    hst = hpool.tile([P, 512], BF16)
                    nc.sync.dma_start(out=hst, in_=hsT_r[k][:, tsl])
                    st, sp = (k == 0), (k == NK - 1)
                    for h in range(HPC):
                        mm(q_ps[h][:], wq_sb[:, k, h * D:(h + 1) * D], hst[:], st, sp)
                    mm(k_ps[:], wk_sb[:, k, :], hst[:], st, sp)
                    mm(v_ps[:], wv_sb[:, k, :], hst[:], st, sp)
                    if k == 2 and pend_vt is not None:
                        pvt, pb4 = pend_vt
                        for si in range(4):
                            pt = ps1t.tile([P, P], BF16, tag="tps")
                            nc.tensor.transpose(pt[:], pvt[:, si * P:(si + 1) * P],
                                                ident[:])
                            nc.vector.tensor_copy(v_sb[:, pb4 * 4 + si, :], pt[:])
                        pend_vt = None
                for h in range(HPC):
                    nc.vector.tensor_copy(qt_sb[:, h, tsl], q_ps[h][:])
                nc.vector.tensor_copy(kt_sb[:, tsl], k_ps[:])
                vt_sb = spool.tile([P, 512], BF16, tag="vt")
                nc.vector.tensor_copy(vt_sb[:], v_ps[:])
                pend_vt = (vt_sb, tb4)
            pvt, pb4 = pend_vt
            for si in range(4):
                pt = ps1t.tile([P, P], BF16, tag="tps")
                nc.tensor.transpose(pt[:], pvt[:, si * P:(si + 1) * P], ident[:])
                nc.vector.tensor_copy(v_sb[:, pb4 * 4 + si, :], pt[:])

        # ---------------- Phase 2: causal attention, 4 heads at once ----
        # S^T tile per (query 128-block tb, key tile st<=tb):
        #   [s=128, (h=4, t=128)] = lhsT(K^T s-tile) @ rhs(Q^T all heads)
        # Denominators: bf16 DVE accumulation of exp tiles over st, then one
        # GpSimd partition_all_reduce per tb (sums over s AND broadcasts the
        # result to all partitions), reciprocal on DVE, scale on DVE.
        dmask_b = dmask[:, None, :].to_broadcast([P, HPC, P])
        with tc.tile_pool(name="ps2s", bufs=3, space="PSUM") as ps2s, \
             tc.tile_pool(name="ps2o", bufs=3, space="PSUM") as ps2o, \
             tc.tile_pool(name="ps3", bufs=2, space="PSUM") as ps3:
            def out_proj(tb):
                """Partial out-projection for query tile tb (emitted one tb
                late so the PE never waits on the normalization chain)."""
                tsl = slice(tb * P, (tb + 1) * P)
                for jb in range(4):
                    jsl = slice(jb * 512, (jb + 1) * 512)
                    op_ps = ps3.tile([P, 512], F32, tag="op")
                    for h in range(HPC):
                        mm(op_ps[:], at_sb[:, h, tsl], wo_sb[:, h, jsl],
                           h == 0, h == HPC - 1)
                    oto = opool.tile([P, 512], BF16, tag="oto")
                    if jb % 2 == 0:
                        nc.vector.tensor_copy(oto[:], op_ps[:])
                    else:
                        nc.scalar.activation(oto[:], op_ps[:], COPY)
                    nc.sync.dma_start(out=out_r[tb][:, jsl], in_=oto[:])

            def finish_row(tb, ot_ps, den):
                """Reciprocal + normalization for row tb — emitted two rows
                late so neither the PE nor the DVE queue ever waits on the
                3.5us GpSimd reduce (whose engine backlog can reach a full
                row during the short early rows)."""
                rec = spool.tile([P, HPC, P], F32, tag="rec")
                nc.vector.reciprocal_approx_fast(out=rec[:], in_=den[:])
                nc.vector.tensor_mul(at_sb[:, :, tb * P:(tb + 1) * P],
                                     ot_ps[:], rec[:])

            from collections import deque
            pend = deque()
            for tb in range(KT):
                tsl = slice(tb * P, (tb + 1) * P)
                qrhs = qt_sb[:, :, tsl]              # [128, 4, 128]
                ot_ps = ps2o.tile([P, HPC, P], F32, tag="ot")
                acc = apool.tile([P, HPC, P], BF16, tag="acc")
                # diagonal tile LAST: the row's first attV then depends only
                # on ACT (exp), never on the DVE mask-multiply — the DVE queue
                # (which the scheduler stalls on the reduce-dependent recip)
                # stays off the PE's row-entry critical path. The ~0.9us it
                # adds to acc-readiness is absorbed by the 2-row out-proj lag.
                order = list(range(tb)) + [tb]
                for i, st in enumerate(order):
                    first, last = (i == 0), (i == len(order) - 1)
                    s_ps = ps2s.tile([P, HPC, P], F32, tag="sps")
                    mm(s_ps[:], kt_sb[:, st * P:(st + 1) * P], qrhs, True, True)
                    e_sb = epool.tile([P, HPC, P], BF16, tag="etile")
                    nc.scalar.activation(e_sb[:], s_ps[:], EXP)
                    if st == tb:  # diagonal tile: causal mask
                        nc.vector.tensor_mul(e_sb[:], e_sb[:], dmask_b)
                    mm(ot_ps[:], v_sb[:, st, :], e_sb[:], first, last)
                    if first:
                        nc.vector.tensor_copy(acc[:], e_sb[:])
                    else:
                        nc.vector.tensor_add(acc[:], acc[:], e_sb[:])
                den = dpool.tile([P, HPC, P], F32, tag="den")
                nc.gpsimd.partition_all_reduce(den[:], acc[:], channels=P,
                                               reduce_op=bass_isa.ReduceOp.add)
                pend.append((tb, ot_ps, den))
                if len(pend) > 2:
                    row = pend.popleft()
                    finish_row(*row)
                    out_proj(row[0])
            while pend:
                row = pend.popleft()
                finish_row(*row)
                out_proj(row[0])

    nc.compile()
    return nc


_CACHE = {}


def _get_program():
    if "nc" not in _CACHE:
        _CACHE["nc"] = _build_program()
    return _CACHE["nc"]


def _host_inputs(hidden_states, Wq, Wk, Wv, Wo):
    """Fold rope+scale into weights, build per-core input maps."""
    import ml_dtypes
    bf16 = ml_dtypes.bfloat16
    f64 = np.float64
    mats = _rope_fold()
    scale = D ** -0.5
    Wq_f = np.empty((HID, HID), dtype=np.float32)
    for h in range(H):
        Wq_f[h * D:(h + 1) * D] = (mats[h] @ Wq[h * D:(h + 1) * D].astype(f64)
                                   * scale).astype(np.float32)
    perm = np.concatenate([np.arange(0, 64, 2), np.arange(1, 64, 2),
                           np.arange(64, 128, 2), np.arange(65, 128, 2)])
    Wk_f = Wk[perm].astype(np.float32)

    wkT = np.ascontiguousarray(Wk_f.T).astype(bf16)
    wvT = np.ascontiguousarray(Wv.T).astype(bf16)
    ii = np.arange(P)[:, None]
    jj = np.arange(P)[None, :]
    dmask = (ii <= jj).astype(bf16)

    hsT = [np.ascontiguousarray(hidden_states[b].T).astype(bf16)
           for b in range(B)]
    in_maps = []
    for c in range(NCORES):
        b, q = c // CPB, c % CPB
        rows = slice(q * HD_PC, (q + 1) * HD_PC)
        in_maps.append({
            "hsT": hsT[b],
            "wqT": np.ascontiguousarray(Wq_f[rows].T).astype(bf16),
            "wkT": wkT,
            "wvT": wvT,
            "woT": np.ascontiguousarray(Wo[:, rows].T).astype(bf16),
            "dmask": dmask,
        })
    return in_maps


def kernel(hidden_states, Wq, Wk, Wv, Wo):
    hidden_states = np.asarray(hidden_states, dtype=np.float32)
    Wq = np.asarray(Wq, dtype=np.float32)
    Wk = np.asarray(Wk, dtype=np.float32)
    Wv = np.asarray(Wv, dtype=np.float32)
    Wo = np.asarray(Wo, dtype=np.float32)

    nc = _get_program()
    in_maps = _host_inputs(hidden_states, Wq, Wk, Wv, Wo)
    res = run_bass_kernel_spmd(nc, in_maps, list(range(NCORES)))
    parts = [np.asarray(r["out"], dtype=np.float32) for r in res.results]
    out = np.empty((B, T, HID), dtype=np.float32)
    for b in range(B):
        out[b] = parts[CPB * b]
        for q in range(1, CPB):
            out[b] += parts[CPB * b + q]
    return out
